# revision 1
# baseline (speedup 1.0000x reference)
"""Trainium2 Bass kernel v3: host feeds BOTH x layouts in bf16.

xt [BP, ND, 128, T] (d-major) feeds the e-matmul directly (no PE
transposes); xc [BP, T, D] (natural, t-major) feeds the context matmul.
8MB/core total, all plain bf16 DMAs (no casting), split over the sync
HWDGE ring (xt + weights) and the gpsimd SWDGE ring (xc). Every on-chip
op pattern is the HW-proven one from the 55us baseline / v1 runs.
"""

import numpy as np

B, T, D, H = 32, 1024, 512, 10
NCORES = 8
BP = B // NCORES
NT = T // 128
ND = D // 128

_CACHE = {}


def _build_nc():
    from contextlib import ExitStack

    import concourse.bass as bass
    import concourse.mybir as mybir
    import concourse.tile as tile
    from concourse import bacc

    f32 = mybir.dt.float32
    bf16 = mybir.dt.bfloat16

    nc = bacc.Bacc("TRN2", target_bir_lowering=False, debug=False, num_devices=NCORES)

    xt_d = nc.dram_tensor("xt", [BP, ND, 128, T], bf16, kind="ExternalInput")
    xc_d = nc.dram_tensor("xc", [BP, T, D], bf16, kind="ExternalInput")
    rb_d = nc.dram_tensor("rb", [H, BP], f32, kind="ExternalInput")
    b2_d = nc.dram_tensor("b2", [1], f32, kind="ExternalInput")
    w1ab_d = nc.dram_tensor("w1ab", [128, ND, H], bf16, kind="ExternalInput")
    w2b_d = nc.dram_tensor("w2b", [H, 1], bf16, kind="ExternalInput")
    out_d = nc.dram_tensor("ctx_out", [BP, D], f32, kind="ExternalOutput")

    with tile.TileContext(nc) as tc, ExitStack() as ctx:
        consts = ctx.enter_context(tc.tile_pool(name="consts", bufs=1))
        xbp = ctx.enter_context(tc.tile_pool(name="xbp", bufs=BP))
        xtp = ctx.enter_context(tc.tile_pool(name="xtp", bufs=BP))
        ep = ctx.enter_context(tc.tile_pool(name="ep", bufs=2))
        sp = ctx.enter_context(tc.tile_pool(name="sp", bufs=2))
        op = ctx.enter_context(tc.tile_pool(name="op", bufs=2))
        pwm = ctx.enter_context(tc.tile_pool(name="pwm", bufs=1, space="PSUM"))
        pe_ps = ctx.enter_context(tc.tile_pool(name="pe_ps", bufs=2, space="PSUM"))
        psm = ctx.enter_context(tc.tile_pool(name="psm", bufs=2, space="PSUM"))
        pcx = ctx.enter_context(tc.tile_pool(name="pcx", bufs=2, space="PSUM"))

        # ---- x loads first: xt halves on sync (HWDGE), xc on gpsimd ----
        xt_tiles = []
        xc_tiles = []
        rb = w1a_b = w2s_b = b2s = None
        for b in range(BP):
            xt = xtp.tile([128, ND, T], bf16, tag="xt", name=f"xt{b}")
            src = xt_d[b].rearrange("J p t -> p J t")
            nc.sync.dma_start(out=xt[:, :, 0:512], in_=src[:, :, 0:512])
            if b == 0:
                rb = consts.tile([H, BP], f32)
                nc.sync.dma_start(out=rb, in_=rb_d[:, :])
                w1a_b = consts.tile([128, ND, H], bf16)
                nc.sync.dma_start(out=w1a_b, in_=w1ab_d[:, :, :])
                w2s_b = consts.tile([H, 1], bf16)
                nc.sync.dma_start(out=w2s_b, in_=w2b_d[:, :])
                b2s = consts.tile([128, 1], f32)
                b2_ap = b2_d[:]
                b2_bcast = bass.AP(
                    tensor=b2_ap.tensor,
                    offset=b2_ap.offset,
                    ap=[[0, 128]] + list(b2_ap.ap),
                )
                nc.sync.dma_start(out=b2s, in_=b2_bcast)
            nc.sync.dma_start(out=xt[:, :, 512:1024], in_=src[:, :, 512:1024])
            xt_tiles.append(xt)
        for b in range(BP):
            xc = xbp.tile([128, NT, D], bf16, tag="xc", name=f"xc{b}")
            srcc = xc_d[b].rearrange("(i p) d -> p i d", p=128)
            nc.gpsimd.dma_start(out=xc[:, 0:4, :], in_=srcc[:, 0:4, :])
            nc.gpsimd.dma_start(out=xc[:, 4:8, :], in_=srcc[:, 4:8, :])
            xc_tiles.append(xc)

        ones = consts.tile([128, 1], bf16)
        nc.vector.memset(ones, 1.0)

        # ---- PE pre-warm while waiting for the first x data ----
        wset = consts.tile([128, 128], bf16)
        nc.vector.memset(wset, 0.0)
        warm_ps = pwm.tile([128, 128], f32, tag="warm", name="warm_ps")
        for _ in range(20):
            nc.tensor.matmul(warm_ps, wset, wset, start=True, stop=True)

        e_tiles = [None] * BP
        expE_tiles = [None] * BP
        den_tiles = [None] * BP
        cps_tiles = [None] * BP

        def e_mm(b):
            # straight 4-long accumulation runs per T-half (d-major xt)
            e_tiles[b] = ep.tile([H, T], bf16, tag="e", name=f"e{b}")
            for h in range(2):
                eps = pe_ps.tile([H, 512], f32, tag="eps", name=f"eps{b}_{h}")
                for J in range(ND):
                    nc.tensor.matmul(
                        eps,
                        w1a_b[:, J, :],
                        xt_tiles[b][:, J, h * 512 : (h + 1) * 512],
                        start=(J == 0),
                        stop=(J == ND - 1),
                    )
                nc.scalar.activation(
                    e_tiles[b][:, h * 512 : (h + 1) * 512],
                    eps,
                    mybir.ActivationFunctionType.Tanh,
                    bias=rb[:, b : b + 1],
                    scale=1.0,
                )

        def energies(b, h):
            if h == 0:
                expE_tiles[b] = (
                    psm.tile([128, NT], f32, tag="small", name=f"en{b}"),
                    sp.tile([128, NT], bf16, tag="exps", name=f"exps{b}"),
                    sp.tile([128, NT], bf16, tag="expE", name=f"expE{b}"),
                )
            enps, exps, expE = expE_tiles[b]
            for q in range(4):
                i = h * 4 + q
                nc.tensor.matmul(
                    enps[:, i : i + 1],
                    e_tiles[b][:, i * 128 : (i + 1) * 128],
                    w2s_b,
                    start=True,
                    stop=True,
                )
            sl = slice(h * 4, (h + 1) * 4)
            nc.scalar.activation(
                exps[:, sl],
                enps[:, sl],
                mybir.ActivationFunctionType.Exp,
                bias=b2s[:, 0:1],
                scale=1.0,
            )
            nc.vector.tensor_scalar_max(expE[:, sl], exps[:, sl], 1.0)

        def denominator(b):
            expE = expE_tiles[b][2]
            dps = psm.tile([1, NT], f32, tag="small", name=f"dps{b}")
            nc.tensor.matmul(dps, ones, expE, start=True, stop=True)
            den = sp.tile([1, 2], f32, tag="den", name=f"den{b}")
            nc.vector.reduce_sum(out=den[:, 0:1], in_=dps, axis=mybir.AxisListType.X)
            nc.vector.reciprocal(den[:, 1:2], den[:, 0:1])
            den_tiles[b] = den

        def context(b):
            cps = pcx.tile([1, D], f32, tag="cps", name=f"cps{b}")
            expE = expE_tiles[b][2]
            for i in range(NT):
                nc.tensor.matmul(
                    cps,
                    expE[:, i : i + 1],
                    xc_tiles[b][:, i, :],
                    start=(i == 0),
                    stop=(i == NT - 1),
                )
            cps_tiles[b] = cps

        def finish(b):
            ctx_sb = op.tile([1, D], f32, tag="ctx", name=f"ctx{b}")
            nc.vector.tensor_scalar_mul(ctx_sb, cps_tiles[b], den_tiles[b][:, 1:2])
            nc.sync.dma_start(out=out_d[b : b + 1, :], in_=ctx_sb)

        # ---- pipelined: ctx(b-1) fills tanh(b) window; e_mm(b+1) fills
        # exp/max(b) window ----
        e_mm(0)
        for b in range(BP):
            if b > 0:
                context(b - 1)
                finish(b - 1)
            energies(b, 0)
            energies(b, 1)
            denominator(b)
            if b + 1 < BP:
                e_mm(b + 1)
        context(BP - 1)
        finish(BP - 1)

    nc.compile()
    return nc


def _get_nc():
    if "nc" not in _CACHE:
        _CACHE["nc"] = _build_nc()
    return _CACHE["nc"]


def _make_in_maps(cbhg, rnn, w1, b1, w2, b2):
    import ml_dtypes

    bf16 = ml_dtypes.bfloat16
    w1b = np.asarray(w1[D:], dtype=np.float64)
    w1ab = np.ascontiguousarray(
        w1[:D].reshape(ND, 128, H).transpose(1, 0, 2).astype(bf16)
    )
    w2b = np.ascontiguousarray(w2.astype(bf16))
    xc_all = np.ascontiguousarray(cbhg.astype(bf16))
    xt_all = np.ascontiguousarray(
        xc_all.reshape(B, T, ND, 128).transpose(0, 2, 3, 1)
    )
    maps = []
    for c in range(NCORES):
        rnn_c = np.ascontiguousarray(rnn[c * BP : (c + 1) * BP])
        rbm = (rnn_c.astype(np.float64) @ w1b + b1.astype(np.float64)).T
        maps.append(
            {
                "xt": np.ascontiguousarray(xt_all[c * BP : (c + 1) * BP]),
                "xc": np.ascontiguousarray(xc_all[c * BP : (c + 1) * BP]),
                "rb": np.ascontiguousarray(rbm.astype(np.float32)),
                "b2": b2,
                "w1ab": w1ab,
                "w2b": w2b,
            }
        )
    return maps


def _run(in_maps, trace=False):
    from concourse.bass_utils import run_bass_kernel_spmd

    nc = _get_nc()
    return run_bass_kernel_spmd(nc, in_maps, core_ids=list(range(NCORES)), trace=trace)


def kernel(cbhg_encoding, attention_rnn_output, W1, b1, W2, b2):
    cbhg = np.asarray(cbhg_encoding, dtype=np.float32)
    rnn = np.asarray(attention_rnn_output, dtype=np.float32)
    w1 = np.ascontiguousarray(np.asarray(W1, dtype=np.float32))
    b1v = np.ascontiguousarray(np.asarray(b1, dtype=np.float32))
    w2 = np.ascontiguousarray(np.asarray(W2, dtype=np.float32))
    b2v = np.ascontiguousarray(np.asarray(b2, dtype=np.float32))

    res = _run(_make_in_maps(cbhg, rnn, w1, b1v, w2, b2v))
    context = np.concatenate(
        [res.results[c]["ctx_out"][:, None, :] for c in range(NCORES)], axis=0
    ).astype(np.float32)
    rnn_reshaped = rnn.reshape(B, 1, D).copy()
    return (context, rnn_reshaped)



# revision 9
# speedup vs baseline: 1.0571x; 1.0571x over previous
"""Trainium2 Bass kernel v4: single-copy x load + DVE context reduction.

x is loaded ONCE per core (4MB bf16, d-major xt layout) instead of the
v3 dual-layout 8MB load. The context contraction over t runs on the
vector engine as fused multiply+accumulate (scalar_tensor_tensor with
accum_out) against the resident xt tiles, so no t-major copy is needed.
Softmax weights are computed as a [1, T] row (e -> energies -> exp),
max-folded (exp(relu(z)) == max(exp(z), 1)) with the denominator taken
for free via accum_out, then partition-broadcast to [128, T] with a
stride-0 SBUF->SBUF DMA on the Act HWDGE ring. Output is the
unnormalized context + per-batch denominator; the host divides.
"""

import numpy as np

B, T, D, H = 32, 1024, 512, 10
NCORES = 8
BP = B // NCORES
NT = T // 128
ND = D // 128

_CACHE = {}


def _build_nc():
    from contextlib import ExitStack

    import concourse.bass as bass
    import concourse.mybir as mybir
    import concourse.tile as tile
    from concourse import bacc

    f32 = mybir.dt.float32
    bf16 = mybir.dt.bfloat16
    Alu = mybir.AluOpType
    Act = mybir.ActivationFunctionType

    nc = bacc.Bacc("TRN2", target_bir_lowering=False, debug=False, num_devices=NCORES)

    xt_d = nc.dram_tensor("xt", [BP, ND, 128, T], bf16, kind="ExternalInput")
    cb_d = nc.dram_tensor("cb", [128, 48], bf16, kind="ExternalInput")
    cf_d = nc.dram_tensor("cf", [128, 8], f32, kind="ExternalInput")
    ctx_d = nc.dram_tensor("ctx_out", [128, BP * ND], f32, kind="ExternalOutput")
    den_d = nc.dram_tensor("den_out", [1, BP], f32, kind="ExternalOutput")
    ab_d = nc.dram_tensor("ab_buf", [BP, 1024], bf16, kind="ExternalOutput")

    with tile.TileContext(nc) as tc, ExitStack() as ctx:
        consts = ctx.enter_context(tc.tile_pool(name="consts", bufs=1))
        xsb = ctx.enter_context(tc.tile_pool(name="xsb", bufs=BP))
        absb = ctx.enter_context(tc.tile_pool(name="absb", bufs=2))
        esb = ctx.enter_context(tc.tile_pool(name="esb", bufs=2))
        asb = ctx.enter_context(tc.tile_pool(name="asb", bufs=2))
        outp = ctx.enter_context(tc.tile_pool(name="outp", bufs=1))
        pe = ctx.enter_context(tc.tile_pool(name="pe", bufs=2, space="PSUM"))
        pen = ctx.enter_context(tc.tile_pool(name="pen", bufs=2, space="PSUM"))

        # ---- input DMAs: sync ring carries x0..x2, act ring consts + x3 ----
        cb = consts.tile([128, 48], bf16)
        nc.sync.dma_start(out=cb, in_=cb_d[:, :])
        cf = consts.tile([128, 8], f32)
        nc.sync.dma_start(out=cf, in_=cf_d[:, :])

        xt_tiles = [None] * BP
        for b in range(BP):
            xt = xsb.tile([128, ND, T], bf16, tag="xt", name=f"xt{b}")
            src = xt_d[b].rearrange("J p t -> p J t")
            eng = nc.sync
            eng.dma_start(out=xt[:, :, 0:512], in_=src[:, :, 0:512])
            eng.dma_start(out=xt[:, :, 512:1024], in_=src[:, :, 512:1024])
            xt_tiles[b] = xt

        junk = consts.tile([128, 1024], bf16)
        nc.vector.memset(junk[:, 0:512], 0.0)
        wz = consts.tile([128, 10], bf16)
        nc.vector.memset(wz, 0.0)

        ctxr = outp.tile([128, BP * ND], f32)
        dent = outp.tile([1, BP], f32)

        # ---- PE pre-warm while the first x DMA is in flight ----
        for _ in range(10):
            wps = pe.tile([10, 1024], f32, tag="pe", name="warm")
            nc.tensor.matmul(wps[:, 0:512], wz, junk[:, 0:512], start=True, stop=True)

        e_ps = [None] * BP
        e_t = [None] * BP
        en_ps = [None] * BP
        a_t = [None] * BP
        ab_t = [None] * BP

        def e_mm(b):
            # e_pre[h, t] accumulated over d-chunks J; J-outer so each
            # weight load serves both t-halves
            e_ps[b] = pe.tile([10, 1024], f32, tag="pe", name=f"eps{b}")
            for J in range(ND):
                for h in range(2):
                    nc.tensor.matmul(
                        e_ps[b][:, h * 512 : (h + 1) * 512],
                        cb[:, J * 10 : (J + 1) * 10],
                        xt_tiles[b][:, J, h * 512 : (h + 1) * 512],
                        start=(J == 0),
                        stop=(J == ND - 1),
                    )

        def tanh(b):
            e_t[b] = esb.tile([10, 1024], bf16, tag="e", name=f"e{b}")
            nc.scalar.activation(
                e_t[b], e_ps[b], Act.Tanh, bias=cf[0:10, b : b + 1], scale=1.0
            )

        def en_mm(b):
            en_ps[b] = pen.tile([1, 1024], f32, tag="pen", name=f"en{b}")
            for h in range(2):
                nc.tensor.matmul(
                    en_ps[b][:, h * 512 : (h + 1) * 512],
                    cb[0:10, 40:41],
                    e_t[b][:, h * 512 : (h + 1) * 512],
                    start=True,
                    stop=True,
                )

        def exp_max(b):
            # a = exp(z + b2); weights = max(a, 1) == exp(relu(z + b2));
            # accum_out gives the softmax denominator for free
            a_t[b] = asb.tile([1, 1024], bf16, tag="a", name=f"a{b}")
            nc.scalar.activation(
                a_t[b], en_ps[b], Act.Exp, bias=cf[0:1, 4:5], scale=1.0
            )
            am = asb.tile([1, 1024], bf16, tag="am", name=f"am{b}")
            nc.vector.tensor_scalar(
                am, a_t[b], 1.0, None, Alu.max, Alu.add, accum_out=dent[:, b : b + 1]
            )
            a_t[b] = am

        def bcast(b):
            # partition-replicate via DRAM bounce: store the [1, 1024] row,
            # then load it back with a stride-0 partition dim (b2 pattern)
            ab_t[b] = absb.tile([128, 1024], bf16, tag="ab", name=f"ab{b}")
            nc.sync.dma_start(out=ab_d[b : b + 1, :], in_=a_t[b][0:1, :])
            s = ab_d[b, :]
            src = bass.AP(tensor=s.tensor, offset=s.offset, ap=[[0, 128]] + list(s.ap))
            nc.sync.dma_start(out=ab_t[b][:, :], in_=src)

        def ctx_mm(b):
            for J in range(ND):
                nc.vector.scalar_tensor_tensor(
                    out=junk,
                    in0=xt_tiles[b][:, J, :],
                    scalar=1.0,
                    in1=ab_t[b],
                    op0=Alu.mult,
                    op1=Alu.mult,
                    accum_out=ctxr[:, b * ND + J : b * ND + J + 1],
                )

        # ---- pipeline: batch order matches DMA arrival (sync: 0,1,2; act: 3) ----
        S = [0, 3, 1, 2]
        e_mm(S[0])
        tanh(S[0])
        e_mm(S[1])
        en_mm(S[0])
        exp_max(S[0])
        bcast(S[0])
        tanh(S[1])
        e_mm(S[2])
        en_mm(S[1])
        exp_max(S[1])
        bcast(S[1])
        ctx_mm(S[0])
        tanh(S[2])
        e_mm(S[3])
        en_mm(S[2])
        exp_max(S[2])
        bcast(S[2])
        ctx_mm(S[1])
        tanh(S[3])
        en_mm(S[3])
        exp_max(S[3])
        bcast(S[3])
        ctx_mm(S[2])
        ctx_mm(S[3])

        nc.sync.dma_start(out=den_d[:, :], in_=dent)
        nc.sync.dma_start(out=ctx_d[:, :], in_=ctxr)

    nc.compile()
    return nc


def _get_nc():
    if "nc" not in _CACHE:
        _CACHE["nc"] = _build_nc()
    return _CACHE["nc"]


def _make_in_maps(cbhg, rnn, w1, b1, w2, b2):
    import ml_dtypes

    bf16 = ml_dtypes.bfloat16
    w1b = np.asarray(w1[D:], dtype=np.float64)
    xt_all = np.ascontiguousarray(
        cbhg.astype(bf16).reshape(B, T, ND, 128).transpose(0, 2, 3, 1)
    )
    cb = np.zeros((128, 48), dtype=bf16)
    cb[:, 0:40] = w1[:D].reshape(ND, 128, H).transpose(1, 0, 2).reshape(128, 40)
    cb[0:10, 40] = w2[:, 0]
    maps = []
    for c in range(NCORES):
        rnn_c = np.asarray(rnn[c * BP : (c + 1) * BP], dtype=np.float64)
        rbm = (rnn_c @ w1b + b1.astype(np.float64)).T
        cf = np.zeros((128, 8), dtype=np.float32)
        cf[0:10, 0:BP] = rbm
        cf[0, 4] = b2[0]
        maps.append(
            {
                "xt": np.ascontiguousarray(xt_all[c * BP : (c + 1) * BP]),
                "cb": cb,
                "cf": cf,
            }
        )
    return maps


def _unpack_out(ctx_raw, den_raw):
    """[128, BP*ND] raw context + [1, BP] denominators -> [BP, D] f32."""
    ctx = ctx_raw.reshape(128, BP, ND).transpose(1, 2, 0).reshape(BP, D)
    return (ctx.astype(np.float64) / den_raw.reshape(BP, 1).astype(np.float64)).astype(
        np.float32
    )


def _run(in_maps, trace=False):
    from concourse.bass_utils import run_bass_kernel_spmd

    nc = _get_nc()
    return run_bass_kernel_spmd(nc, in_maps, core_ids=list(range(NCORES)), trace=trace)


def kernel(cbhg_encoding, attention_rnn_output, W1, b1, W2, b2):
    cbhg = np.asarray(cbhg_encoding, dtype=np.float32)
    rnn = np.asarray(attention_rnn_output, dtype=np.float32)
    w1 = np.ascontiguousarray(np.asarray(W1, dtype=np.float32))
    b1v = np.ascontiguousarray(np.asarray(b1, dtype=np.float32))
    w2 = np.ascontiguousarray(np.asarray(W2, dtype=np.float32))
    b2v = np.ascontiguousarray(np.asarray(b2, dtype=np.float32))

    res = _run(_make_in_maps(cbhg, rnn, w1, b1v, w2, b2v))
    context = np.concatenate(
        [
            _unpack_out(res.results[c]["ctx_out"], res.results[c]["den_out"])[:, None, :]
            for c in range(NCORES)
        ],
        axis=0,
    ).astype(np.float32)
    rnn_reshaped = rnn.reshape(B, 1, D).copy()
    return (context, rnn_reshaped)
